# revision 9
# baseline (speedup 1.0000x reference)
"""Trainium2 Bass kernel for nn_Attn (Luong 'general' attention scoring + softmax).

Reference computation:
    energy[s,b,:] = W @ encoder_outputs[s,b,:] + b          # [S,B,H]
    score[b,s]    = hidden[b,:] . energy[s,b,:]             # [B,S]
    attn          = softmax(score, axis=s)[:, None, :]      # [B,1,S]

Algebraic restructuring (exact up to fp reassociation):
    score[b,s] = (W^T hidden[b]) . enc[s,b] + hidden[b].b_vec
The bias term is constant over s, so it cancels in the softmax. Hence:
    u = hidden @ W                  # [B,H]  (tiny matmul)
    score[b,s] = u[b] . enc[s,b]    # streaming dot over H   (memory-bound)
    attn = softmax_s(score)

Sharding: data-parallel over batch B=32 across 8 cores (4 rows each); W
replicated. No cross-core communication (softmax is per-b over s).

Per-core pipeline:
  - DMA W (8 chunks) + hidden; PE: transpose hidden, u = hidden @ W,
    broadcast u[b] to 128 partitions (ones-vector matmul).
  - Main loop (64 tiles of [128s x 1024h]): DMA enc tile; DVE
    tensor_tensor_reduce computes (enc * U_b) and row-reduce -> score column.
  - Softmax: per-b max via free-reduce + PE transpose + free-reduce;
    exp with accum (ACT) for sums; total via ones-matmul partition reduce;
    attn = exp(score - max - ln(sum)) applied on PE-transposed scores so the
    result lands in [16,128] layout = contiguous s-order for the output DMA.
"""

import numpy as np

import concourse.bacc as bacc
import concourse.mybir as mybir
import concourse.tile as tile
from concourse.bass_utils import run_bass_kernel_spmd

S, B, H = 2048, 32, 1024
NCORES = 8
BS = B // NCORES          # 4 batch rows per core
P = 128                   # partitions
KC = H // P               # 8 contraction chunks
NCH = S // P              # 16 score chunks per b
F32 = mybir.dt.float32

_CACHED = {}


def _build_program():
    nc = bacc.Bacc("TRN2", target_bir_lowering=False, debug=False)

    hid_d = nc.dram_tensor("hidden", [BS, H], F32, kind="ExternalInput")
    enc_d = nc.dram_tensor("enc", [S, BS, H], F32, kind="ExternalInput")
    w_d = nc.dram_tensor("w", [H, H], F32, kind="ExternalInput")
    idt_d = nc.dram_tensor("ident", [P, P], F32, kind="ExternalInput")
    ones_d = nc.dram_tensor("ones", [P, P], F32, kind="ExternalInput")
    # sel[k, b*P + m] = 1 if k == b else 0 — as matmul lhsT it replicates
    # row b of the rhs across all 128 output partitions (base partition 0).
    sel_d = nc.dram_tensor("sel", [BS, BS * P], F32, kind="ExternalInput")
    out_d = nc.dram_tensor("out", [BS, S], F32, kind="ExternalOutput")

    AF = mybir.ActivationFunctionType
    ALU = mybir.AluOpType

    with tile.TileContext(nc) as tc:
        with (
            tc.tile_pool(name="const", bufs=1) as cpool,
            tc.tile_pool(name="wpool", bufs=2) as wpool,
            tc.tile_pool(name="enc", bufs=12) as epool,
            tc.tile_pool(name="scr", bufs=2) as spool,
            tc.tile_pool(name="psum", bufs=1, space="PSUM") as psum,
        ):
            idt = cpool.tile([P, P], F32, tag="idt")
            nc.sync.dma_start(idt[:], idt_d[:])
            ones = cpool.tile([P, P], F32, tag="ones")
            nc.sync.dma_start(ones[:], ones_d[:])
            hid = cpool.tile([BS, H], F32, tag="hid")
            nc.sync.dma_start(hid[:], hid_d[:])
            sel = cpool.tile([BS, BS * P], F32, tag="sel")
            nc.sync.dma_start(sel[:], sel_d[:])

            # hidden^T chunks: [BS, 128] -> [128, BS]
            hT = []
            for k in range(KC):
                pt = psum.tile([P, BS], F32, tag="mm", bufs=2)
                nc.tensor.transpose(
                    pt[:], hid[:, k * P:(k + 1) * P], idt[0:BS, 0:BS]
                )
                t = cpool.tile([P, BS], F32, tag=f"hT{k}")
                nc.scalar.copy(t[:], pt[:])
                hT.append(t)

            # u = hidden @ W   [BS, H], accumulated over KC chunks in PSUM
            u_sb = cpool.tile([BS, H], F32, tag="u")
            pu0 = psum.tile([BS, 512], F32, tag="pu0")
            pu1 = psum.tile([BS, 512], F32, tag="pu1")
            for k in range(KC):
                wc = wpool.tile([P, H], F32, tag="w")
                nc.sync.dma_start(wc[:], w_d[k * P:(k + 1) * P, :])
                for j, pu in enumerate((pu0, pu1)):
                    nc.tensor.matmul(
                        pu[:], hT[k][:], wc[:, j * 512:(j + 1) * 512],
                        start=(k == 0), stop=(k == KC - 1),
                    )
            nc.scalar.copy(u_sb[:, 0:512], pu0[:])
            nc.scalar.copy(u_sb[:, 512:1024], pu1[:])

            # broadcast u[b,:] to all 128 partitions: U_b [128, H]
            Ub = []
            for b in range(BS):
                t = cpool.tile([P, H], F32, tag=f"U{b}")
                for j in range(2):
                    pb = psum.tile([P, 512], F32, tag="mm", bufs=2)
                    nc.tensor.matmul(
                        pb[:], sel[:, b * P:(b + 1) * P],
                        u_sb[:, j * 512:(j + 1) * 512],
                        start=True, stop=True,
                    )
                    nc.scalar.copy(t[:, j * 512:(j + 1) * 512], pb[:])
                Ub.append(t)

            # main loop: score[b][p, c] for s = c*128 + p
            scores = [cpool.tile([P, NCH], F32, tag=f"sc{b}", name=f"sc{b}") for b in range(BS)]
            for b in range(BS):
                for c in range(NCH):
                    et = epool.tile([P, H], F32, tag="et")
                    nc.sync.dma_start(et[:], enc_d[c * P:(c + 1) * P, b, :])
                    prod = spool.tile([P, H], F32, tag="prod", name="prod")
                    nc.vector.tensor_mul(prod[:], et[:], Ub[b][:])
                    scr = spool.tile([P, H], F32, tag="scr", name="scr")
                    nc.scalar.activation(
                        scr[:], prod[:], AF.Copy,
                        accum_out=scores[b][:, c:c + 1],
                    )

            # ---- softmax over s (per b) ----
            # per-partition max over chunks, then reduce across partitions
            rmax = cpool.tile([P, BS], F32, tag="rmax")
            for b in range(BS):
                nc.vector.tensor_reduce(
                    rmax[:, b:b + 1], scores[b][:],
                    axis=mybir.AxisListType.X, op=ALU.max,
                )
            prt = psum.tile([BS, P], F32, tag="mm", bufs=2)
            nc.tensor.transpose(prt[:], rmax[:], idt[:])
            rmaxT = cpool.tile([BS, P], F32, tag="rmaxT")
            nc.scalar.copy(rmaxT[:], prt[:])
            gmax = cpool.tile([BS, 1], F32, tag="gmax")
            nc.vector.tensor_reduce(
                gmax[:], rmaxT[:], axis=mybir.AxisListType.X, op=ALU.max
            )

            # bias1 = broadcast(-gmax) -> [128, BS]
            negg = cpool.tile([BS, 1], F32, tag="negg")
            nc.scalar.mul(negg[:], gmax[:], -1.0)
            png = psum.tile([1, BS], F32, tag="mm", bufs=2)
            nc.tensor.transpose(png[:], negg[:], idt[0:BS, 0:BS])
            nggT = cpool.tile([1, BS], F32, tag="nggT")
            nc.scalar.copy(nggT[:], png[:])
            pb1 = psum.tile([P, BS], F32, tag="mm", bufs=2)
            nc.tensor.matmul(pb1[:], ones[0:1, :], nggT[:], start=True, stop=True)
            bias1 = cpool.tile([P, BS], F32, tag="bias1")
            nc.scalar.copy(bias1[:], pb1[:])

            # exp(score - gmax) partial sums per partition, then total per b
            partials = cpool.tile([P, BS], F32, tag="partials")
            for b in range(BS):
                scr2 = spool.tile([P, NCH], F32, tag="scr2")
                nc.scalar.activation(
                    scr2[:], scores[b][:], AF.Exp,
                    bias=bias1[:, b:b + 1],
                    accum_out=partials[:, b:b + 1],
                )
            pT = psum.tile([BS, 1], F32, tag="mm", bufs=2)
            nc.tensor.matmul(pT[:], partials[:], ones[:, 0:1], start=True, stop=True)
            T4 = cpool.tile([BS, 1], F32, tag="T4")
            nc.scalar.copy(T4[:], pT[:])
            lnT = cpool.tile([BS, 1], F32, tag="lnT")
            nc.scalar.activation(lnT[:], T4[:], AF.Ln)

            # bias2 = broadcast(-(gmax + lnT)) -> [128, BS]
            b2 = cpool.tile([BS, 1], F32, tag="b2")
            nc.vector.tensor_add(b2[:], gmax[:], lnT[:])
            b2n = cpool.tile([BS, 1], F32, tag="b2n")
            nc.scalar.mul(b2n[:], b2[:], -1.0)
            pb2t = psum.tile([1, BS], F32, tag="mm", bufs=2)
            nc.tensor.transpose(pb2t[:], b2n[:], idt[0:BS, 0:BS])
            b2T = cpool.tile([1, BS], F32, tag="b2T")
            nc.scalar.copy(b2T[:], pb2t[:])
            pb2 = psum.tile([P, BS], F32, tag="mm", bufs=2)
            nc.tensor.matmul(pb2[:], ones[0:1, :], b2T[:], start=True, stop=True)
            bias2 = cpool.tile([P, BS], F32, tag="bias2")
            nc.scalar.copy(bias2[:], pb2[:])

            # attn = exp(score - gmax - lnT), emitted in transposed [16,128]
            # layout == contiguous s-order per b, then one DMA per b.
            for b in range(BS):
                pst = psum.tile([NCH, P], F32, tag="mm", bufs=2)
                nc.tensor.transpose(pst[:], scores[b][:], idt[:])
                ob = spool.tile([NCH, P], F32, tag="ob")
                nc.scalar.activation(
                    ob[:], pst[:], AF.Exp, bias=bias2[0:NCH, b:b + 1]
                )
                nc.sync.dma_start(
                    out_d[b, :].rearrange("(c p) -> c p", p=P), ob[:]
                )

    nc.compile()
    return nc


def _get_program():
    if "nc" not in _CACHED:
        _CACHED["nc"] = _build_program()
    return _CACHED["nc"]


def _run(hidden, encoder_outputs, W, **spmd_kwargs):
    nc = _get_program()
    hidden = np.ascontiguousarray(np.asarray(hidden, dtype=np.float32))
    enc = np.asarray(encoder_outputs, dtype=np.float32)
    W = np.ascontiguousarray(np.asarray(W, dtype=np.float32))
    ident = np.eye(P, dtype=np.float32)
    ones = np.ones((P, P), dtype=np.float32)
    sel = np.zeros((BS, BS * P), dtype=np.float32)
    for k in range(BS):
        sel[k, k * P:(k + 1) * P] = 1.0

    in_maps = []
    for i in range(NCORES):
        bs = slice(BS * i, BS * (i + 1))
        in_maps.append({
            "hidden": np.ascontiguousarray(hidden[bs]),
            "enc": np.ascontiguousarray(enc[:, bs, :]),
            "w": W,
            "ident": ident,
            "ones": ones,
            "sel": sel,
        })

    res = run_bass_kernel_spmd(
        nc, in_maps, core_ids=list(range(NCORES)), **spmd_kwargs
    )
    out = np.concatenate([r["out"] for r in res.results], axis=0)
    return out[:, None, :].astype(np.float32), res


def kernel(hidden, encoder_outputs, W, b):
    out, _ = _run(hidden, encoder_outputs, W)
    return out
